# revision 81
# baseline (speedup 1.0000x reference)
"""GCN message-passing kernel for 8 Trainium2 NeuronCores (Bass/Tile).

Computes (matching the jax reference):
    h = x @ W_conv                      [N, H]
    node_embed = leaky_relu(D^-1/2 (A+I) D^-1/2 h + b_conv)
    out = sigmoid(leaky(cat(e[i], e[j]) @ W1 + b1) @ W2 + b2)

Distribution: nodes dst-sharded over the 8 cores. The scaled features
g = dinv * h are exchanged with two chunked AllGathers (one per shard
half) so per-edge source gathers can start as soon as the first chunk
lands. Edges are packed into pooled per-(group,bucket) chunk streams
(chunks may span destination tiles; boundary chunks get one matmul per
tile) and scatter-added on the TensorEngine via one-hot matmuls.
Self-loop contributions are added locally from the resident g tiles.
The pair-MLP head reuses the same pooled gather/permute machinery on
the chunked e AllGather.
"""

import re

import numpy as np

import concourse.bass as bass
import concourse.bacc as bacc
import concourse.mybir as mybir
import concourse.tile as tile
from concourse import library_config
from concourse.bass_utils import run_bass_kernel_spmd

NC = 8
N_NODES = 100000
F_IN = 256
H = 64
NEG = 0.01

P = 128                    # partitions / tile height
TILES = 98                 # node tiles per core
SHARD = TILES * P          # 12544 nodes per core
NPAD = NC * SHARD          # 100352
HTILES = 49                # tiles per AllGather chunk
HSHARD = HTILES * P        # 6272
NBUCKET = 4                # edge: (src core half) x (src parity)
GROUP = 8                  # node tiles per edge window group



def _wrap_idx_window(idx):
    """int array [W] (W % 16 == 0) -> [128, W//16] int16 wrapped/replicated."""
    w = idx.reshape(-1, 16).T.astype(np.int16)
    return np.tile(w, (8, 1))


def _node_bucket(n, splits):
    """node id -> (bucket, pair-row in that bucket's table) for an
    AllGather chunking of each core's tiles into `splits` (tile counts)."""
    c = n // SHARD
    off = n % SHARD
    bases = np.concatenate([[0], np.cumsum(splits)]) * P
    a = (np.searchsorted(bases, off, side="right") - 1).astype(np.int64)
    sizes = np.asarray(splits, np.int64) * P
    row = c * sizes[a] + off - bases[a]
    par = n & 1
    return a * 2 + par, row >> 1


def _build_onehot(loc_arr):
    """loc_arr [NC, totunits, P(row)] -> fp8 one-hot [NC, P(row), units, P(col)]."""
    import ml_dtypes
    cols = np.arange(P, dtype=np.int64)
    oh = (loc_arr[:, :, :, None] == cols).astype(ml_dtypes.float8_e4m3)
    return np.ascontiguousarray(oh.transpose(0, 2, 1, 3))


def _pooled_sched(core, tl, loc, bucket, prow, ntiles, group_sz,
                  nbucket=NBUCKET):
    """Pooled chunk-stream schedule.

    Items (one per scatter row): destination (core, tile tl, column loc),
    gather source (bucket, prow). Rows are packed per (core, window)
    where window = (tile group, bucket); chunks of 128 rows may span
    tiles -> boundary chunks get one matmul unit per covered tile.
    Unit/chunk structure is shared across cores (max-padded); pad rows
    are trailing -1 indices (SWDGE trims them) with loc=255.
    """
    items = len(core)
    ngroups = (ntiles + group_sz - 1) // group_sz
    grp = tl // group_sz
    tloc = tl - grp * group_sz
    win = grp * nbucket + bucket
    nwin = ngroups * nbucket

    cnt = np.zeros((NC, nwin), np.int64)
    np.add.at(cnt, (core, win), 1)
    K = np.maximum(1, -(-cnt.max(axis=0) // P))        # chunks per window
    woff = np.concatenate([[0], np.cumsum(K)])          # chunk offsets
    totchunks = int(K.sum())
    totidx = totchunks * P

    cnt_t = np.zeros((NC, nwin, group_sz), np.int64)
    np.add.at(cnt_t, (core, win, tloc), 1)
    cum = np.cumsum(cnt_t, axis=2) - cnt_t              # tile start offsets

    units = [set() for _ in range(nwin)]
    for w in range(nwin):
        g = w // nbucket
        tcount = min(group_sz, ntiles - g * group_sz)
        for c in range(NC):
            for tt in range(tcount):
                s, e = cum[c, w, tt], cum[c, w, tt] + cnt_t[c, w, tt]
                if e == s:
                    continue
                for ci in range(s // P, (e - 1) // P + 1):
                    units[w].add((ci, tt))
    units = [sorted(u) for u in units]
    # every tile must appear in >=1 unit per bucket-PAIR (each AG chunk's
    # bucket pair may be consumed as a separate accumulation phase)
    for g in range(ngroups):
        tcount = min(group_sz, ntiles - g * group_sz)
        for half in range(nbucket // 2):
            present = set()
            for b in (2 * half, 2 * half + 1):
                present.update(tt for (_, tt) in units[g * nbucket + b])
            missing = [tt for tt in range(tcount) if tt not in present]
            if missing:
                w0 = g * nbucket + 2 * half
                units[w0].extend((0, tt) for tt in missing)
                units[w0].sort()
    ulen = [len(u) for u in units]
    uoff = np.concatenate([[0], np.cumsum(ulen)]).astype(np.int64)
    totunits = int(uoff[-1])

    kmax = int(K.max())
    lut = np.full((nwin, kmax, group_sz), -1, np.int64)
    for w in range(nwin):
        for i, (ci, tt) in enumerate(units[w]):
            lut[w, ci, tt] = uoff[w] + i

    order = np.lexsort((tl, win, core))
    so_core = core[order]
    so_win = win[order]
    so_tloc = tloc[order]
    so_loc = loc[order]
    so_prow = prow[order]
    key = so_core * nwin + so_win
    starts = np.r_[0, np.flatnonzero(np.diff(key)) + 1]
    run_ids = np.zeros(items, np.int64)
    run_ids[starts[1:]] = 1
    run_ids = np.cumsum(run_ids)
    rank = np.arange(items) - starts[run_ids]
    ci = rank // P
    rr = rank % P
    u = lut[so_win, ci, so_tloc]
    assert (u >= 0).all()

    PAD_TRIM = False
    idx_lin = np.full((NC, totidx), -1 if PAD_TRIM else 0, np.int64)
    loc_arr = np.full((NC, totunits, P), 255, np.int64)
    idx_lin[so_core, (woff[so_win] + ci) * P + rr] = so_prow
    loc_arr[so_core, u, rr] = so_loc

    idx_i16 = np.zeros((NC, P, totidx // 16), np.int16)
    for w in range(nwin):
        lo, hi = woff[w] * P, (woff[w] + K[w]) * P
        for c in range(NC):
            idx_i16[c][:, lo // 16: hi // 16] = _wrap_idx_window(idx_lin[c, lo:hi])
    loc_f16 = np.ascontiguousarray(
        loc_arr.transpose(0, 2, 1)).astype(np.float16)

    sched = {
        "ntiles": ntiles,
        "group_sz": group_sz,
        "ngroups": ngroups,
        "nbucket": nbucket,
        "K": K,
        "woff": woff,
        "uoff": uoff,
        "units": units,
        "totchunks": totchunks,
        "totidx": totidx,
        "totunits": totunits,
        "_dbg": (idx_lin, loc_arr),
    }
    return sched, idx_i16, loc_f16


def _emit_flags(sched, phases):
    """flags[(w, i)] = (start, stop) for emitted matmuls: first/last unit
    per tile within each phase (a phase = a list of windows emitted as one
    PSUM accumulation pass)."""
    flags = {}
    nbucket = sched["nbucket"]
    for win_order in phases:
        seen = {}
        for w in win_order:
            g = w // nbucket
            for i, (ci, tt) in enumerate(sched["units"][w]):
                t = g * sched["group_sz"] + tt
                seen.setdefault(t, []).append((w, i))
        for t, lst in seen.items():
            for j, wi in enumerate(lst):
                flags[wi] = (j == 0, j == len(lst) - 1)
    return flags


def _prep(inputs):
    x = np.asarray(inputs["x"], np.float32)
    edge_index = np.asarray(inputs["edge_index"], np.int64)
    index = np.asarray(inputs["index"], np.int64)
    W_conv = np.asarray(inputs["W_conv"], np.float32)
    b_conv = np.asarray(inputs["b_conv"], np.float32)
    W1 = np.asarray(inputs["W1"], np.float32)
    b1 = np.asarray(inputs["b1"], np.float32)
    W2 = np.asarray(inputs["W2"], np.float32)
    b2 = np.asarray(inputs["b2"], np.float32)

    n = x.shape[0]
    src = edge_index[0].astype(np.int64)
    dst = edge_index[1].astype(np.int64)

    # degrees include self-loops (loops handled locally on-device)
    deg = np.bincount(dst, minlength=NPAD).astype(np.float32)
    deg += 1.0
    deg[n:] = 1.0

    # edge buckets: (src core half) x parity — contiguous halves of g_full
    ghalf = (src >= (NC // 2) * SHARD).astype(np.int64)
    ebucket = ghalf * 2 + (src & 1)
    eprow = (src - ghalf * (NPAD // 2)) >> 1
    esched, eidx, eloc = _pooled_sched(
        core=dst // SHARD, tl=(dst % SHARD) // P, loc=dst % P,
        bucket=ebucket, prow=eprow, ntiles=TILES, group_sz=GROUP)
    esched["table"] = "corehalf"

    eoh = _build_onehot(esched["_dbg"][1])

    # ---- pair head: z-partial exchange (no e AllGather) ----
    # every slot (pair, side) is served by the core owning its node: that
    # core gathers the e row LOCALLY, computes z = e_row @ W1side, and the
    # tiny z tables are AllGathered; pairs are then assembled per
    # destination core with one-hot accumulation matmuls.
    B = index.shape[0]
    PB = B // NC
    assert PB % P == 0
    PCH = PB // P
    s_pair = np.concatenate([np.arange(B), np.arange(B)]).astype(np.int64)
    s_side = np.concatenate([np.zeros(B, np.int64), np.ones(B, np.int64)])
    s_node = np.concatenate([index[:, 0], index[:, 1]]).astype(np.int64)
    z_owner = s_node // SHARD
    off = s_node % SHARD
    zbucket = s_side * 2 + (s_node & 1)          # (W1 side) x (row parity)
    zrow = off >> 1                               # local pair-packed row
    ZNB = 4
    zcnt = np.zeros((NC, ZNB), np.int64)
    np.add.at(zcnt, (z_owner, zbucket), 1)
    zK = -(-zcnt.max(axis=0) // P)                # chunks per bucket
    znidx = zK * P
    zboff = np.concatenate([[0], np.cumsum(znidx)])
    ZS = int(zboff[-1])                           # z rows per core

    order = np.lexsort((s_pair, zbucket, z_owner))
    so_owner = z_owner[order]
    so_bucket = zbucket[order]
    so_row = zrow[order]
    so_pair = s_pair[order]
    key = so_owner * ZNB + so_bucket
    starts = np.r_[0, np.flatnonzero(np.diff(key)) + 1]
    run_ids = np.zeros(len(key), np.int64)
    run_ids[starts[1:]] = 1
    run_ids = np.cumsum(run_ids)
    rank = np.arange(len(key)) - starts[run_ids]
    pos = so_owner * ZS + zboff[so_bucket] + rank  # position in zall

    zidx_lin = np.zeros((NC, ZS), np.int64)
    zidx_lin[so_owner, zboff[so_bucket] + rank] = so_row
    zidx = np.zeros((NC, P, ZS // 16), np.int16)
    for b in range(ZNB):
        lo, hi = int(zboff[b]), int(zboff[b + 1])
        for c in range(NC):
            zidx[c][:, lo // 16: hi // 16] = _wrap_idx_window(zidx_lin[c, lo:hi])

    # assembly: gather 256B-packed zall rows (8 z-slots each) per dest core
    # and scatter-add into pair chunks via the pooled one-hot machinery;
    # bucket = sub-slot within the packed row
    dest = so_pair // PB
    pchunk = (so_pair % PB) // P
    pcol = so_pair % P
    asched, aidx, aloc = _pooled_sched(
        core=dest, tl=pchunk, loc=pcol,
        bucket=pos % 8, prow=pos // 8,
        ntiles=PCH, group_sz=8, nbucket=8)
    aoh = _build_onehot(asched["_dbg"][1])

    zsched = {"ZNB": ZNB, "zK": zK, "znidx": znidx, "zboff": zboff,
              "ZS": ZS, "PCH": PCH, "_dbg": (zidx_lin, pos, order)}

    xpad = np.zeros((NPAD, F_IN), np.float32)
    xpad[:n] = x
    xT = xpad.T.astype(np.float16)
    xT_shards = [
        np.ascontiguousarray(
            xT[:, c * SHARD:(c + 1) * SHARD].reshape(2, P, SHARD).transpose(1, 0, 2)
        ) for c in range(NC)
    ]
    deg_sb = [
        np.ascontiguousarray(
            deg[c * SHARD:(c + 1) * SHARD].reshape(TILES, P).T
        ) for c in range(NC)
    ]

    # w1z[p, s, :] = W1[s*64 + p%64]: lhsT/rhs partition alignment for any
    # (side, parity) combination in the z-compute matmuls
    w1z = np.tile(W1.reshape(2, 64, 16).transpose(1, 0, 2), (2, 1, 1))
    consts = {
        "wc": np.ascontiguousarray(
            W_conv.reshape(2, P, H).transpose(1, 0, 2)).astype(np.float16),
        "bconvb": np.broadcast_to(b_conv, (P, H)).astype(np.float32).copy(),
        "w1z": np.ascontiguousarray(w1z).astype(np.float16),
        "ident": np.eye(P, dtype=np.float16),
        "b1b": np.broadcast_to(b1, (P, 16)).astype(np.float32).copy(),
        "w2b": np.broadcast_to(W2.reshape(16), (P, 16)).astype(np.float32).copy(),
        "b2b": np.broadcast_to(b2.reshape(1), (P, 1)).astype(np.float32).copy(),
    }
    sched = {"edge": esched, "z": zsched, "asm": asched, "PCH": PCH}
    in_maps = []
    for c in range(NC):
        m = {
            "xt": xT_shards[c],
            "deg": deg_sb[c],
            "eidx": eidx[c],
            "zidx": zidx[c],
            "aidx": aidx[c],
            "eoh": eoh[c],
            "aoh": aoh[c],
        }
        m.update(consts)
        in_maps.append(m)
    return in_maps, sched


def _build(sched):
    dt = mybir.dt
    esched = sched["edge"]
    zsched = sched["z"]
    asched = sched["asm"]
    PCH = sched["PCH"]
    ZS = zsched["ZS"]

    nc = bacc.Bacc("TRN2", target_bir_lowering=False, debug=False,
                   enable_asserts=False, num_devices=NC, num_swdge_queues=4)

    xt_in = nc.dram_tensor("xt", [P, 2, SHARD], dt.float16, kind="ExternalInput")
    deg_in = nc.dram_tensor("deg", [P, TILES], dt.float32, kind="ExternalInput")
    eidx_in = nc.dram_tensor("eidx", [P, esched["totidx"] // 16], dt.int16,
                             kind="ExternalInput")
    zidx_in = nc.dram_tensor("zidx", [P, ZS // 16], dt.int16,
                             kind="ExternalInput")
    aidx_in = nc.dram_tensor("aidx", [P, asched["totidx"] // 16], dt.int16,
                             kind="ExternalInput")
    eoh_in = nc.dram_tensor("eoh", [P, esched["totunits"], P], dt.float8e4,
                            kind="ExternalInput")
    aoh_in = nc.dram_tensor("aoh", [P, asched["totunits"], P], dt.float8e4,
                            kind="ExternalInput")
    wc_in = nc.dram_tensor("wc", [P, 2, H], dt.float16, kind="ExternalInput")
    bconvb_in = nc.dram_tensor("bconvb", [P, H], dt.float32, kind="ExternalInput")
    w1z_in = nc.dram_tensor("w1z", [P, 2, 16], dt.float16, kind="ExternalInput")
    ident_in = nc.dram_tensor("ident", [P, P], dt.float16, kind="ExternalInput")
    b1b_in = nc.dram_tensor("b1b", [P, 16], dt.float32, kind="ExternalInput")
    w2b_in = nc.dram_tensor("w2b", [P, 16], dt.float32, kind="ExternalInput")
    b2_in = nc.dram_tensor("b2b", [P, 1], dt.float32, kind="ExternalInput")
    outp = nc.dram_tensor("out", [PCH * P, 1], dt.float32, kind="ExternalOutput")

    g_shard = nc.dram_tensor("g_shard", [SHARD, H], dt.float16)
    g_full = nc.dram_tensor("g_full", [NPAD, H], dt.float16, addr_space="Shared")
    e_shard = nc.dram_tensor("e_shard", [SHARD, H], dt.float16)
    zsend = nc.dram_tensor("zsend", [ZS, 16], dt.float16)
    zall = nc.dram_tensor("zall", [NC * ZS, 16], dt.float16, addr_space="Shared")

    gf_pairs = g_full[:, :].rearrange("(r two) f -> r (two f)", two=2)
    gA_pairs = [gf_pairs[:NPAD // 4, :], gf_pairs[NPAD // 4:, :]]
    es_pairs = e_shard[:, :].rearrange("(r two) f -> r (two f)", two=2)
    za_packed = zall[:, :].rearrange("(r eight) f -> r (eight f)", eight=8)

    # rotate bucket emission per group; the extra g//4 step breaks the
    # lag-(msg bufs) resonance where the same bucket is always emitted last
    def erot(g):
        r = (g + g // 4) % NBUCKET
        return [(r + j) % NBUCKET for j in range(NBUCKET)]

    def arot(g):
        return [(g + j) % 8 for j in range(8)]

    ephase = [g * NBUCKET + b for g in range(esched["ngroups"]) for b in erot(g)]
    eflags = _emit_flags(esched, [ephase])
    aphase = [g * 8 + b for g in range(asched["ngroups"]) for b in arot(g)]
    aflags = _emit_flags(asched, [aphase])

    with tile.TileContext(nc) as tc:
        nc.gpsimd.load_library(library_config.mlp)

        with (
            tc.tile_pool(name="const", bufs=1) as cpool,
            tc.tile_pool(name="dinvp", bufs=1) as dpool,
            tc.tile_pool(name="gres", bufs=1) as gpool,
        ):
            wc_sb = cpool.tile([P, 2, H], dt.float16)
            nc.sync.dma_start(wc_sb[:], wc_in[:, :, :])
            bconvb = cpool.tile([P, H], dt.float32)
            nc.sync.dma_start(bconvb[:], bconvb_in[:, :])
            w1z_sb = cpool.tile([P, 2, 16], dt.float16)
            nc.sync.dma_start(w1z_sb[:], w1z_in[:, :, :])
            ident = cpool.tile([P, P], dt.float16)
            nc.sync.dma_start(ident[:], ident_in[:, :])
            b1b_sb = cpool.tile([P, 16], dt.float32)
            nc.sync.dma_start(b1b_sb[:], b1b_in[:, :])
            w2b_sb = cpool.tile([P, 16], dt.float32)
            nc.sync.dma_start(w2b_sb[:], w2b_in[:, :])
            b2_sb = cpool.tile([P, 1], dt.float32)
            nc.sync.dma_start(b2_sb[:], b2_in[:, :])
            eidx_sb = cpool.tile([P, esched["totidx"] // 16], dt.int16)
            nc.sync.dma_start(eidx_sb[:], eidx_in[:, :])
            zidx_sb = cpool.tile([P, ZS // 16], dt.int16)
            nc.sync.dma_start(zidx_sb[:], zidx_in[:, :])
            aidx_sb = cpool.tile([P, asched["totidx"] // 16], dt.int16)
            nc.sync.dma_start(aidx_sb[:], aidx_in[:, :])

            deg_sb = dpool.tile([P, TILES], dt.float32)
            nc.sync.dma_start(deg_sb[:], deg_in[:, :])
            sq = dpool.tile([P, TILES], dt.float32)
            nc.scalar.activation(sq[:], deg_sb[:], mybir.ActivationFunctionType.Sqrt)
            dinv = dpool.tile([P, TILES], dt.float32)
            nc.vector.reciprocal(dinv[:], sq[:])

            g_sb = gpool.tile([P, TILES, H], dt.float16)
            g2_sb = gpool.tile([P, TILES, H], dt.float16)
            e_sb = gpool.tile([P, TILES, H], dt.float16)

            # ---------------- phase A: g = (x @ W) * dinv ----------------
            with (
                tc.tile_pool(name="xtp", bufs=2) as xtp,
                tc.tile_pool(name="hps", bufs=4, space="PSUM") as hps,
            ):
                blocks = [(0, 16), (16, 32), (32, 49),
                          (49, 65), (65, 81), (81, TILES)]
                for bi, (t0, t1) in enumerate(blocks):
                    xt_sb = xtp.tile([P, 2, (t1 - t0) * P], dt.float16, tag="xt")
                    nc.sync.dma_start(xt_sb[:], xt_in[:, :, t0 * P: t1 * P])
                    for t in range(t0, t1):
                        h_ps = hps.tile([P, H], dt.float32)
                        for k in range(2):
                            nc.tensor.matmul(
                                h_ps[:],
                                lhsT=xt_sb[:, k, (t - t0) * P:(t - t0 + 1) * P],
                                rhs=wc_sb[:, k, :],
                                start=(k == 0), stop=(k == 1),
                            )
                        nc.vector.tensor_scalar(
                            g_sb[:, t, :], h_ps[:], dinv[:, t:t + 1], None,
                            mybir.AluOpType.mult,
                        )
                    # write this block's g slice out immediately so the
                    # AllGather can trigger right after the last block
                    nc.sync.dma_start(
                        g_shard[t0 * P:t1 * P, :].rearrange(
                            "(t p) f -> p t f", p=P),
                        g_sb[:, t0:t1, :],
                    )
                nc.gpsimd.collective_compute(
                    "AllGather", mybir.AluOpType.bypass,
                    replica_groups=[list(range(NC))],
                    ins=[g_shard[:, :].opt()],
                    outs=[g_full[:, :].opt()],
                )
                # G2 = g*dinv + b_conv (consume becomes acc*dinv + G2);
                # computed under the AllGather, off the critical path
                for t in range(TILES):
                    nc.vector.tensor_scalar(
                        g2_sb[:, t, :], g_sb[:, t, :], dinv[:, t:t + 1],
                        None, mybir.AluOpType.mult,
                    )
                    nc.vector.tensor_tensor(
                        g2_sb[:, t, :], g2_sb[:, t, :], bconvb[:],
                        mybir.AluOpType.add,
                    )

            # ---------------- phase C: aggregate per dst tile ----------------
            EMSG_BUFS = 2
            with (
                tc.tile_pool(name="emsg", bufs=EMSG_BUFS) as msgp,
                tc.tile_pool(name="eoh", bufs=2) as ohp,
                tc.tile_pool(name="epost", bufs=4) as postp,
                tc.tile_pool(name="pmsg", bufs=1) as pmsgp,
                tc.tile_pool(name="poh", bufs=2) as pohp,
                tc.tile_pool(name="zpool", bufs=1) as zpool,
                tc.tile_pool(name="zrtp", bufs=2) as zrtp,
            ):
                ngroups_e = esched["ngroups"]
                ngroups_a = asched["ngroups"]

                # zero msg buffers once: gathers trim trailing -1 pad rows,
                # leaving those slots as-is, so they must never hold NaN bits
                for b in range(NBUCKET):
                    kmax = max(int(esched["K"][g * NBUCKET + b])
                               for g in range(ngroups_e))
                    for _ in range(EMSG_BUFS):
                        mt = msgp.tile([P, kmax, P], dt.float16, tag=f"em{b}")
                        nc.vector.memset(mt[:], 0)
                for b in range(8):
                    kmax = max(int(asched["K"][ag * 8 + b])
                               for ag in range(ngroups_a))
                    mt = pmsgp.tile([P, kmax, P], dt.float16, tag=f"pm{b}")
                    nc.vector.memset(mt[:], 0)

                def emit_window(sched_, idx_sb, oh_in, tabs, flags,
                                pools, acc_tiles, w, prefix, rhs_w):
                    mp, op_, ap = pools
                    nb = sched_["nbucket"]
                    g = w // nb
                    b = w % nb
                    K = int(sched_["K"][w])
                    nidx = K * P
                    ioff = int(sched_["woff"][w]) * P
                    uoff = int(sched_["uoff"][w])
                    nu = len(sched_["units"][w])
                    sel = (b & 1) if prefix == "e" else b
                    tab = tabs[b // 2] if prefix == "e" else tabs[0]
                    msg = mp.tile([P, K, P], dt.float16, tag=f"{prefix}m{b}",
                                  name=f"{prefix}msg_w{w}")
                    # split large windows into two gathers: descriptors only
                    # trigger at gen end, so half 1 drains under half 2's gen
                    halves = [(0, K // 2), (K // 2, K)] if K >= 8 else [(0, K)]
                    for (k0, k1) in halves:
                        if k0 == k1:
                            continue
                        hn = (k1 - k0) * P
                        hoff = ioff + k0 * P
                        nc.gpsimd.dma_gather(
                            msg[:, k0:k1, :], tab,
                            idx_sb[:, hoff // 16:(hoff + hn) // 16],
                            hn, hn, P, single_packet=False, queue_num=0)
                    if nu == 0:
                        return
                    oh = op_.tile([P, nu, P], dt.float8e4,
                                  tag=f"{prefix}o{b}" if prefix == "e"
                                  else f"po{b & 1}",
                                  name=f"{prefix}oh_w{w}")
                    nc.sync.dma_start(oh[:], oh_in[:, uoff:uoff + nu, :])
                    for i, (ci, tt) in enumerate(sched_["units"][w]):
                        t = g * sched_["group_sz"] + tt
                        if t not in acc_tiles:
                            acc_tiles[t] = ap.tile(
                                [P, rhs_w], dt.float32,
                                tag=f"a{t % sched_['group_sz']}",
                                name=f"{prefix}acc_t{t}_w{w}")
                        st, sp = flags[(w, i)]
                        nc.tensor.matmul(
                            acc_tiles[t][:],
                            lhsT=oh[:, i, :],
                            rhs=msg[:, ci, sel * rhs_w:(sel + 1) * rhs_w],
                            start=st, stop=sp,
                        )

                def consume_edge(t, a):
                    # e = leaky(acc*dinv + G2); all-DVE to avoid cross-engine
                    # sem round-trips inside the serialized vector FIFO
                    s1 = postp.tile([P, H], dt.float32, tag="e1", name=f"e1_{t}")
                    nc.vector.tensor_scalar(
                        s1[:], a[:], dinv[:, t:t + 1], None, mybir.AluOpType.mult)
                    nc.vector.tensor_tensor(
                        s1[:], s1[:], g2_sb[:, t, :], mybir.AluOpType.add)
                    m = postp.tile([P, H], dt.float32, tag="m", name=f"m_{t}")
                    nc.vector.tensor_scalar(
                        m[:], s1[:], NEG, None, mybir.AluOpType.mult)
                    nc.vector.tensor_tensor(
                        e_sb[:, t, :], s1[:], m[:], mybir.AluOpType.max)

                with tc.tile_pool(name="eacc", bufs=1, space="PSUM") as accp:
                    epools = (msgp, ohp, accp)
                    apools = (pmsgp, pohp, accp)

                    for g in range(ngroups_e):
                        acc_tiles = {}
                        for b in erot(g):
                            emit_window(esched, eidx_sb, eoh_in, gA_pairs,
                                        eflags, epools, acc_tiles,
                                        g * NBUCKET + b, "e", H)
                        t0 = g * GROUP
                        t1 = min(t0 + GROUP, TILES)
                        for t in range(t0, t1):
                            consume_edge(t, acc_tiles.pop(t))
                        # stream this group's e rows out right away
                        nc.sync.dma_start(
                            e_shard[t0 * P:t1 * P, :].rearrange(
                                "(t p) f -> p t f", p=P),
                            e_sb[:, t0:t1, :],
                        )

                    # ------- z-partials from LOCAL e (no e AllGather) -------
                    zK = zsched["zK"]
                    zboff = zsched["zboff"]
                    z_sb = zpool.tile([P, ZS // P, 16], dt.float16)
                    znmax = max(int(v) for v in zsched["znidx"])
                    for b in range(4):
                        side, par = b // 2, b & 1
                        nidx = int(zsched["znidx"][b])
                        zrt = zrtp.tile([P, znmax // P, P], dt.float16,
                                        tag="zr", name=f"zrt{b}")
                        nc.gpsimd.dma_gather(
                            zrt[:, :nidx // P, :], es_pairs,
                            zidx_sb[:, int(zboff[b]) // 16:
                                    (int(zboff[b]) + nidx) // 16],
                            nidx, nidx, P, single_packet=False, queue_num=0)
                        for ci in range(int(zK[b])):
                            tp = accp.tile([P, P], dt.float16,
                                           tag=f"a{(2 * ci) % 8}",
                                           name=f"ztp_{b}_{ci}")
                            nc.tensor.transpose(
                                tp[:], zrt[:, ci, :], ident[:])
                            tps = zrtp.tile([P, P], dt.float16, tag="tps",
                                            name=f"ztps_{b}_{ci}")
                            nc.vector.tensor_copy(tps[:], tp[:])
                            zc = accp.tile([P, 16], dt.float32,
                                           tag=f"a{(2 * ci + 1) % 8}",
                                           name=f"zc_{b}_{ci}")
                            nc.tensor.matmul(
                                zc[:],
                                lhsT=tps[par * 64:(par + 1) * 64, :],
                                rhs=w1z_sb[par * 64:(par + 1) * 64, side, :],
                                start=True, stop=True)
                            nc.vector.tensor_copy(
                                z_sb[:, int(zboff[b]) // P + ci, :], zc[:])
                    nc.sync.dma_start(
                        zsend[:, :].rearrange("(c p) f -> p c f", p=P),
                        z_sb[:, :, :])
                    nc.gpsimd.collective_compute(
                        "AllGather", mybir.AluOpType.bypass,
                        replica_groups=[list(range(NC))],
                        ins=[zsend[:, :].opt()],
                        outs=[zall[:, :].opt()],
                    )

                    # ------- assemble pairs + MLP head -------
                    def consume_pair(t, a):
                        zb = postp.tile([P, 16], dt.float32, tag="zb",
                                        name=f"zb_{t}")
                        nc.vector.tensor_tensor(
                            zb[:], a[:], b1b_sb[:], mybir.AluOpType.add)
                        m2 = postp.tile([P, 16], dt.float32, tag="m2",
                                        name=f"m2_{t}")
                        nc.vector.tensor_scalar(
                            m2[:], zb[:], NEG, None, mybir.AluOpType.mult)
                        z2 = postp.tile([P, 16], dt.float32, tag="z2",
                                        name=f"z2_{t}")
                        nc.vector.tensor_tensor(
                            z2[:], zb[:], m2[:], mybir.AluOpType.max)
                        prod = postp.tile([P, 16], dt.float32, tag="pr",
                                          name=f"pr_{t}")
                        nc.vector.tensor_tensor(
                            prod[:], z2[:], w2b_sb[:], mybir.AluOpType.mult)
                        o_sb = postp.tile([P, 1], dt.float32, tag="ot",
                                          name=f"o_{t}")
                        nc.vector.tensor_reduce(
                            o_sb[:], prod[:], mybir.AxisListType.X,
                            mybir.AluOpType.add)
                        osg = postp.tile([P, 1], dt.float32, tag="os",
                                         name=f"os_{t}")
                        nc.scalar.activation(
                            osg[:], o_sb[:],
                            mybir.ActivationFunctionType.Sigmoid,
                            bias=b2_sb[:, 0:1], scale=1.0)
                        nc.sync.dma_start(outp[t * P:(t + 1) * P, :], osg[:])

                    for ag in range(ngroups_a):
                        acc_tiles = {}
                        for b in arot(ag):
                            emit_window(asched, aidx_sb, aoh_in, [za_packed],
                                        aflags, apools, acc_tiles,
                                        ag * 8 + b, "p", 16)
                        for t in range(ag * 8, min((ag + 1) * 8, PCH)):
                            consume_pair(t, acc_tiles.pop(t))

    # align each gather's SWDGE queue with its Tile-assigned DMA lane so
    # semaphore<->queue locking stays consistent (4-way parallel desc gen)
    for blk in nc.m.functions[0].blocks:
        for inst in blk.instructions:
            if isinstance(inst, mybir.InstDMAGatherAnt):
                si = inst.sync_info
                for u in (si.on_update if si else []):
                    mm = re.match(r"DMASW(\d+)_", u.ant_name or "")
                    if mm:
                        inst.queue_num = int(mm.group(1)) % 4
                        break

    nc.compile()
    return nc


def kernel(**inputs) -> np.ndarray:
    in_maps, sched = _prep(inputs)
    nc = _build(sched)
    res = run_bass_kernel_spmd(nc, in_maps, list(range(NC)))
    out = np.concatenate([res.results[c]["out"] for c in range(NC)], axis=0)
    return out.astype(np.float32)


# revision 83
# speedup vs baseline: 1.1687x; 1.1687x over previous
"""GCN message-passing kernel for 8 Trainium2 NeuronCores (Bass/Tile).

Computes (matching the jax reference):
    h = x @ W_conv                      [N, H]
    node_embed = leaky_relu(D^-1/2 (A+I) D^-1/2 h + b_conv)
    out = sigmoid(leaky(cat(e[i], e[j]) @ W1 + b1) @ W2 + b2)

Distribution: nodes dst-sharded over the 8 cores. The scaled features
g = dinv * h are exchanged with two chunked AllGathers (one per shard
half) so per-edge source gathers can start as soon as the first chunk
lands. Edges are packed into pooled per-(group,bucket) chunk streams
(chunks may span destination tiles; boundary chunks get one matmul per
tile) and scatter-added on the TensorEngine via one-hot matmuls.
Self-loop contributions are added locally from the resident g tiles.
The pair-MLP head reuses the same pooled gather/permute machinery on
the chunked e AllGather.
"""

import re

import numpy as np

import concourse.bass as bass
import concourse.bacc as bacc
import concourse.mybir as mybir
import concourse.tile as tile
from concourse import library_config
from concourse.bass_utils import run_bass_kernel_spmd

NC = 8
N_NODES = 100000
F_IN = 256
H = 64
NEG = 0.01

P = 128                    # partitions / tile height
TILES = 98                 # node tiles per core
SHARD = TILES * P          # 12544 nodes per core
NPAD = NC * SHARD          # 100352
HTILES = 49                # tiles per AllGather chunk
HSHARD = HTILES * P        # 6272
NBUCKET = 4                # edge: (src core half) x (src parity)
GROUP = 8                  # node tiles per edge window group
PGROUP = 8                 # pair slot-tiles per window group
PNBUCKET = 4               # pair: (src core half) x (src parity)


def _wrap_idx_window(idx):
    """int array [W] (W % 16 == 0) -> [128, W//16] int16 wrapped/replicated."""
    w = idx.reshape(-1, 16).T.astype(np.int16)
    return np.tile(w, (8, 1))


def _node_bucket(n, splits):
    """node id -> (bucket, pair-row in that bucket's table) for an
    AllGather chunking of each core's tiles into `splits` (tile counts)."""
    c = n // SHARD
    off = n % SHARD
    bases = np.concatenate([[0], np.cumsum(splits)]) * P
    a = (np.searchsorted(bases, off, side="right") - 1).astype(np.int64)
    sizes = np.asarray(splits, np.int64) * P
    row = c * sizes[a] + off - bases[a]
    par = n & 1
    return a * 2 + par, row >> 1


def _build_onehot(loc_arr):
    """loc_arr [NC, totunits, P(row)] -> fp8 one-hot [NC, P(row), units, P(col)]."""
    import ml_dtypes
    cols = np.arange(P, dtype=np.int64)
    oh = (loc_arr[:, :, :, None] == cols).astype(ml_dtypes.float8_e4m3)
    return np.ascontiguousarray(oh.transpose(0, 2, 1, 3))


def _pooled_sched(core, tl, loc, bucket, prow, ntiles, group_sz,
                  nbucket=NBUCKET):
    """Pooled chunk-stream schedule.

    Items (one per scatter row): destination (core, tile tl, column loc),
    gather source (bucket, prow). Rows are packed per (core, window)
    where window = (tile group, bucket); chunks of 128 rows may span
    tiles -> boundary chunks get one matmul unit per covered tile.
    Unit/chunk structure is shared across cores (max-padded); pad rows
    are trailing -1 indices (SWDGE trims them) with loc=255.
    """
    items = len(core)
    ngroups = (ntiles + group_sz - 1) // group_sz
    grp = tl // group_sz
    tloc = tl - grp * group_sz
    win = grp * nbucket + bucket
    nwin = ngroups * nbucket

    cnt = np.zeros((NC, nwin), np.int64)
    np.add.at(cnt, (core, win), 1)
    K = np.maximum(1, -(-cnt.max(axis=0) // P))        # chunks per window
    woff = np.concatenate([[0], np.cumsum(K)])          # chunk offsets
    totchunks = int(K.sum())
    totidx = totchunks * P

    cnt_t = np.zeros((NC, nwin, group_sz), np.int64)
    np.add.at(cnt_t, (core, win, tloc), 1)
    cum = np.cumsum(cnt_t, axis=2) - cnt_t              # tile start offsets

    units = [set() for _ in range(nwin)]
    for w in range(nwin):
        g = w // nbucket
        tcount = min(group_sz, ntiles - g * group_sz)
        for c in range(NC):
            for tt in range(tcount):
                s, e = cum[c, w, tt], cum[c, w, tt] + cnt_t[c, w, tt]
                if e == s:
                    continue
                for ci in range(s // P, (e - 1) // P + 1):
                    units[w].add((ci, tt))
    units = [sorted(u) for u in units]
    # every tile must appear in >=1 unit per bucket-PAIR (each AG chunk's
    # bucket pair may be consumed as a separate accumulation phase)
    for g in range(ngroups):
        tcount = min(group_sz, ntiles - g * group_sz)
        for half in range(nbucket // 2):
            present = set()
            for b in (2 * half, 2 * half + 1):
                present.update(tt for (_, tt) in units[g * nbucket + b])
            missing = [tt for tt in range(tcount) if tt not in present]
            if missing:
                w0 = g * nbucket + 2 * half
                units[w0].extend((0, tt) for tt in missing)
                units[w0].sort()
    ulen = [len(u) for u in units]
    uoff = np.concatenate([[0], np.cumsum(ulen)]).astype(np.int64)
    totunits = int(uoff[-1])

    kmax = int(K.max())
    lut = np.full((nwin, kmax, group_sz), -1, np.int64)
    for w in range(nwin):
        for i, (ci, tt) in enumerate(units[w]):
            lut[w, ci, tt] = uoff[w] + i

    order = np.lexsort((tl, win, core))
    so_core = core[order]
    so_win = win[order]
    so_tloc = tloc[order]
    so_loc = loc[order]
    so_prow = prow[order]
    key = so_core * nwin + so_win
    starts = np.r_[0, np.flatnonzero(np.diff(key)) + 1]
    run_ids = np.zeros(items, np.int64)
    run_ids[starts[1:]] = 1
    run_ids = np.cumsum(run_ids)
    rank = np.arange(items) - starts[run_ids]
    ci = rank // P
    rr = rank % P
    u = lut[so_win, ci, so_tloc]
    assert (u >= 0).all()

    PAD_TRIM = False
    idx_lin = np.full((NC, totidx), -1 if PAD_TRIM else 0, np.int64)
    loc_arr = np.full((NC, totunits, P), 255, np.int64)
    idx_lin[so_core, (woff[so_win] + ci) * P + rr] = so_prow
    loc_arr[so_core, u, rr] = so_loc

    idx_i16 = np.zeros((NC, P, totidx // 16), np.int16)
    for w in range(nwin):
        lo, hi = woff[w] * P, (woff[w] + K[w]) * P
        for c in range(NC):
            idx_i16[c][:, lo // 16: hi // 16] = _wrap_idx_window(idx_lin[c, lo:hi])
    loc_f16 = np.ascontiguousarray(
        loc_arr.transpose(0, 2, 1)).astype(np.float16)

    sched = {
        "ntiles": ntiles,
        "group_sz": group_sz,
        "ngroups": ngroups,
        "nbucket": nbucket,
        "K": K,
        "woff": woff,
        "uoff": uoff,
        "units": units,
        "totchunks": totchunks,
        "totidx": totidx,
        "totunits": totunits,
        "_dbg": (idx_lin, loc_arr),
    }
    return sched, idx_i16, loc_f16


def _emit_flags(sched, phases):
    """flags[(w, i)] = (start, stop) for emitted matmuls: first/last unit
    per tile within each phase (a phase = a list of windows emitted as one
    PSUM accumulation pass)."""
    flags = {}
    nbucket = sched["nbucket"]
    for win_order in phases:
        seen = {}
        for w in win_order:
            g = w // nbucket
            for i, (ci, tt) in enumerate(sched["units"][w]):
                t = g * sched["group_sz"] + tt
                seen.setdefault(t, []).append((w, i))
        for t, lst in seen.items():
            for j, wi in enumerate(lst):
                flags[wi] = (j == 0, j == len(lst) - 1)
    return flags


def _prep(inputs):
    x = np.asarray(inputs["x"], np.float32)
    edge_index = np.asarray(inputs["edge_index"], np.int64)
    index = np.asarray(inputs["index"], np.int64)
    W_conv = np.asarray(inputs["W_conv"], np.float32)
    b_conv = np.asarray(inputs["b_conv"], np.float32)
    W1 = np.asarray(inputs["W1"], np.float32)
    b1 = np.asarray(inputs["b1"], np.float32)
    W2 = np.asarray(inputs["W2"], np.float32)
    b2 = np.asarray(inputs["b2"], np.float32)

    n = x.shape[0]
    src = edge_index[0].astype(np.int64)
    dst = edge_index[1].astype(np.int64)

    # degrees include self-loops (loops handled locally on-device)
    deg = np.bincount(dst, minlength=NPAD).astype(np.float32)
    deg += 1.0
    deg[n:] = 1.0

    # edge buckets: (src core half) x parity — contiguous halves of g_full
    ghalf = (src >= (NC // 2) * SHARD).astype(np.int64)
    ebucket = ghalf * 2 + (src & 1)
    eprow = (src - ghalf * (NPAD // 2)) >> 1
    esched, eidx, eloc = _pooled_sched(
        core=dst // SHARD, tl=(dst % SHARD) // P, loc=dst % P,
        bucket=ebucket, prow=eprow, ntiles=TILES, group_sz=GROUP)
    esched["table"] = "corehalf"

    # pair stream: per core PB pairs; side slots [xi: 0..PB) [xj: PB..2PB)
    B = index.shape[0]
    PB = B // NC
    assert PB % P == 0
    PCH = PB // P
    pair_global = np.arange(B, dtype=np.int64)
    pcore = pair_global // PB
    plocal = pair_global % PB
    s_core = np.concatenate([pcore, pcore])
    s_slot = np.concatenate([plocal, PB + plocal])
    s_node = np.concatenate([index[:, 0], index[:, 1]]).astype(np.int64)
    phalf = (s_node >= (NC // 2) * SHARD).astype(np.int64)
    pbucket = phalf * 2 + (s_node & 1)
    pprow = (s_node - phalf * (NPAD // 2)) >> 1
    psched, pidx, ploc = _pooled_sched(
        core=s_core, tl=s_slot // P, loc=s_slot % P,
        bucket=pbucket, prow=pprow, ntiles=2 * PCH, group_sz=PGROUP,
        nbucket=PNBUCKET)
    psched["table"] = "corehalf"
    eoh = _build_onehot(esched["_dbg"][1])
    poh = _build_onehot(psched["_dbg"][1])

    xpad = np.zeros((NPAD, F_IN), np.float32)
    xpad[:n] = x
    xT = xpad.T.astype(np.float16)
    xT_shards = [
        np.ascontiguousarray(
            xT[:, c * SHARD:(c + 1) * SHARD].reshape(2, P, SHARD).transpose(1, 0, 2)
        ) for c in range(NC)
    ]
    deg_sb = [
        np.ascontiguousarray(
            deg[c * SHARD:(c + 1) * SHARD].reshape(TILES, P).T
        ) for c in range(NC)
    ]

    consts = {
        "wc": np.ascontiguousarray(
            W_conv.reshape(2, P, H).transpose(1, 0, 2)).astype(np.float16),
        "bconvb": np.broadcast_to(b_conv, (P, H)).astype(np.float32).copy(),
        "ident": np.eye(P, dtype=np.float16),
        "w1": W1.astype(np.float16),
        "b1": b1.reshape(16, 1).astype(np.float32),
        "w2": W2.astype(np.float32),
        "b2t": b2.reshape(1, 1).astype(np.float32),
    }
    sched = {"edge": esched, "pair": psched, "PCH": PCH}
    in_maps = []
    for c in range(NC):
        m = {
            "xt": xT_shards[c],
            "deg": deg_sb[c],
            "eidx": eidx[c],
            "pidx": pidx[c],
            "eoh": eoh[c],
            "poh": poh[c],
        }
        m.update(consts)
        in_maps.append(m)
    return in_maps, sched


def _build(sched):
    dt = mybir.dt
    esched = sched["edge"]
    psched = sched["pair"]
    PCH = sched["PCH"]

    nc = bacc.Bacc("TRN2", target_bir_lowering=False, debug=False,
                   enable_asserts=False, num_devices=NC, num_swdge_queues=4)

    xt_in = nc.dram_tensor("xt", [P, 2, SHARD], dt.float16, kind="ExternalInput")
    deg_in = nc.dram_tensor("deg", [P, TILES], dt.float32, kind="ExternalInput")
    eidx_in = nc.dram_tensor("eidx", [P, esched["totidx"] // 16], dt.int16,
                             kind="ExternalInput")
    pidx_in = nc.dram_tensor("pidx", [P, psched["totidx"] // 16], dt.int16,
                             kind="ExternalInput")
    eoh_in = nc.dram_tensor("eoh", [P, esched["totunits"], P], dt.float8e4,
                            kind="ExternalInput")
    poh_in = nc.dram_tensor("poh", [P, psched["totunits"], P], dt.float8e4,
                            kind="ExternalInput")
    wc_in = nc.dram_tensor("wc", [P, 2, H], dt.float16, kind="ExternalInput")
    bconvb_in = nc.dram_tensor("bconvb", [P, H], dt.float32, kind="ExternalInput")
    ident_in = nc.dram_tensor("ident", [P, P], dt.float16, kind="ExternalInput")
    w1_in = nc.dram_tensor("w1", [P, 16], dt.float16, kind="ExternalInput")
    b1_in = nc.dram_tensor("b1", [16, 1], dt.float32, kind="ExternalInput")
    w2_in = nc.dram_tensor("w2", [16, 1], dt.float32, kind="ExternalInput")
    b2_in = nc.dram_tensor("b2t", [1, 1], dt.float32, kind="ExternalInput")
    outp = nc.dram_tensor("out", [PCH * P, 1], dt.float32, kind="ExternalOutput")

    g_shard = nc.dram_tensor("g_shard", [SHARD, H], dt.float16)
    g_full = nc.dram_tensor("g_full", [NPAD, H], dt.float16, addr_space="Shared")
    e_shard = nc.dram_tensor("e_shard", [SHARD, H], dt.float16)
    e_full = nc.dram_tensor("e_full", [NPAD, H], dt.float16, addr_space="Shared")

    gf_pairs = g_full[:, :].rearrange("(r two) f -> r (two f)", two=2)
    gA_pairs = [gf_pairs[:NPAD // 4, :], gf_pairs[NPAD // 4:, :]]
    ef_pairs = e_full[:, :].rearrange("(r two) f -> r (two f)", two=2)
    eA_pairs = [ef_pairs[:NPAD // 4, :], ef_pairs[NPAD // 4:, :]]

    # rotate bucket emission per group; the extra g//4 step breaks the
    # lag-(msg bufs) resonance where the same bucket is always emitted last
    def erot(g):
        r = (g + g // 4) % NBUCKET
        return [(r + j) % NBUCKET for j in range(NBUCKET)]

    def prot(pg):
        return [(pg + j) % PNBUCKET for j in range(PNBUCKET)]

    ephase = [g * NBUCKET + b for g in range(esched["ngroups"]) for b in erot(g)]
    eflags = _emit_flags(esched, [ephase])
    pphase = [pg * PNBUCKET + b for pg in range(psched["ngroups"])
              for b in prot(pg)]
    pflags = _emit_flags(psched, [pphase])

    with tile.TileContext(nc) as tc:
        nc.gpsimd.load_library(library_config.mlp)

        with (
            tc.tile_pool(name="const", bufs=1) as cpool,
            tc.tile_pool(name="dinvp", bufs=1) as dpool,
            tc.tile_pool(name="gres", bufs=1) as gpool,
        ):
            wc_sb = cpool.tile([P, 2, H], dt.float16)
            nc.sync.dma_start(wc_sb[:], wc_in[:, :, :])
            bconvb = cpool.tile([P, H], dt.float32)
            nc.sync.dma_start(bconvb[:], bconvb_in[:, :])
            ident = cpool.tile([P, P], dt.float16)
            nc.sync.dma_start(ident[:], ident_in[:, :])
            w1_sb = cpool.tile([P, 16], dt.float16)
            nc.sync.dma_start(w1_sb[:], w1_in[:, :])
            b1_sb = cpool.tile([16, 1], dt.float32)
            nc.sync.dma_start(b1_sb[:], b1_in[:, :])
            w2_sb = cpool.tile([16, 1], dt.float32)
            nc.sync.dma_start(w2_sb[:], w2_in[:, :])
            b2_sb = cpool.tile([1, 1], dt.float32)
            nc.sync.dma_start(b2_sb[:], b2_in[:, :])
            deg_sb = dpool.tile([P, TILES], dt.float32)
            nc.sync.dma_start(deg_sb[:], deg_in[:, :])
            sq = dpool.tile([P, TILES], dt.float32)
            nc.scalar.activation(sq[:], deg_sb[:], mybir.ActivationFunctionType.Sqrt)
            dinv = dpool.tile([P, TILES], dt.float32)
            nc.vector.reciprocal(dinv[:], sq[:])

            g_sb = gpool.tile([P, TILES, H], dt.float16)
            g2_sb = gpool.tile([P, TILES, H], dt.float16)
            e_sb = gpool.tile([P, TILES, H], dt.float16)

            # ---------------- phase A: g = (x @ W) * dinv ----------------
            with (
                tc.tile_pool(name="xtp", bufs=2) as xtp,
                tc.tile_pool(name="hps", bufs=4, space="PSUM") as hps,
            ):
                blocks = [(0, 16), (16, 32), (32, 49),
                          (49, 65), (65, 81), (81, TILES)]
                for bi, (t0, t1) in enumerate(blocks):
                    xt_sb = xtp.tile([P, 2, (t1 - t0) * P], dt.float16, tag="xt")
                    nc.sync.dma_start(xt_sb[:], xt_in[:, :, t0 * P: t1 * P])
                    for t in range(t0, t1):
                        h_ps = hps.tile([P, H], dt.float32)
                        for k in range(2):
                            nc.tensor.matmul(
                                h_ps[:],
                                lhsT=xt_sb[:, k, (t - t0) * P:(t - t0 + 1) * P],
                                rhs=wc_sb[:, k, :],
                                start=(k == 0), stop=(k == 1),
                            )
                        nc.vector.tensor_scalar(
                            g_sb[:, t, :], h_ps[:], dinv[:, t:t + 1], None,
                            mybir.AluOpType.mult,
                        )
                    # write this block's g slice out immediately so the
                    # AllGather can trigger right after the last block
                    nc.sync.dma_start(
                        g_shard[t0 * P:t1 * P, :].rearrange(
                            "(t p) f -> p t f", p=P),
                        g_sb[:, t0:t1, :],
                    )
                nc.gpsimd.collective_compute(
                    "AllGather", mybir.AluOpType.bypass,
                    replica_groups=[list(range(NC))],
                    ins=[g_shard[:, :].opt()],
                    outs=[g_full[:, :].opt()],
                )
                # G2 = g*dinv + b_conv (consume becomes acc*dinv + G2);
                # computed under the AllGather, off the critical path
                for t in range(TILES):
                    nc.vector.tensor_scalar(
                        g2_sb[:, t, :], g_sb[:, t, :], dinv[:, t:t + 1],
                        None, mybir.AluOpType.mult,
                    )
                    nc.vector.tensor_tensor(
                        g2_sb[:, t, :], g2_sb[:, t, :], bconvb[:],
                        mybir.AluOpType.add,
                    )

            # big index tables load after phase A's x tiles (Sync is FIFO;
            # the first gather needs them only after the g AllGather)
            eidx_sb = cpool.tile([P, esched["totidx"] // 16], dt.int16)
            nc.sync.dma_start(eidx_sb[:], eidx_in[:, :])
            pidx_sb = cpool.tile([P, psched["totidx"] // 16], dt.int16)
            nc.sync.dma_start(pidx_sb[:], pidx_in[:, :])

            # ---------------- phase C: aggregate per dst tile ----------------
            EMSG_BUFS = 2
            with (
                tc.tile_pool(name="emsg", bufs=EMSG_BUFS) as msgp,
                tc.tile_pool(name="eoh", bufs=2) as ohp,
                tc.tile_pool(name="epost", bufs=4) as postp,
                tc.tile_pool(name="pmsg", bufs=2) as pmsgp,
                tc.tile_pool(name="poh", bufs=2) as pohp,
                tc.tile_pool(name="pxs", bufs=1) as pxsp,
            ):
                xs_sb = pxsp.tile([P, psched["ntiles"], H], dt.float16)
                ngroups_e = esched["ngroups"]
                ngroups_p = psched["ngroups"]

                # zero msg buffers once: gathers trim trailing -1 pad rows,
                # leaving those slots as-is, so they must never hold NaN bits
                for b in range(NBUCKET):
                    kmax = max(int(esched["K"][g * NBUCKET + b])
                               for g in range(ngroups_e))
                    for _ in range(EMSG_BUFS):
                        mt = msgp.tile([P, kmax, P], dt.float16, tag=f"em{b}")
                        nc.vector.memset(mt[:], 0)
                for b in range(PNBUCKET):
                    kmax = max(int(psched["K"][pg * PNBUCKET + b])
                               for pg in range(ngroups_p))
                    for _ in range(2):
                        mt = pmsgp.tile([P, kmax, P], dt.float16, tag=f"pm{b}")
                        nc.vector.memset(mt[:], 0)

                def emit_window(sched_, idx_sb, oh_in, tabs, flags,
                                pools, acc_tiles, w, prefix):
                    mp, op_, ap = pools
                    nb = sched_["nbucket"]
                    g = w // nb
                    b = w % nb
                    K = int(sched_["K"][w])
                    nidx = K * P
                    ioff = int(sched_["woff"][w]) * P
                    uoff = int(sched_["uoff"][w])
                    nu = len(sched_["units"][w])
                    par = b & 1
                    msg = mp.tile([P, K, P], dt.float16, tag=f"{prefix}m{b}",
                                  name=f"{prefix}msg_w{w}")
                    # split large windows into two gathers: descriptors only
                    # trigger at gen end, so half 1 drains under half 2's gen
                    halves = [(0, K // 2), (K // 2, K)] if K >= 8 else [(0, K)]
                    for (k0, k1) in halves:
                        if k0 == k1:
                            continue
                        hn = (k1 - k0) * P
                        hoff = ioff + k0 * P
                        nc.gpsimd.dma_gather(
                            msg[:, k0:k1, :], tabs[b // 2],
                            idx_sb[:, hoff // 16:(hoff + hn) // 16],
                            hn, hn, P, single_packet=False, queue_num=0)
                    if nu == 0:
                        return
                    oh = op_.tile([P, nu, P], dt.float8e4,
                                  tag=f"{prefix}o{b}" if prefix == "e"
                                  else f"po{b & 1}",
                                  name=f"{prefix}oh_w{w}")
                    nc.sync.dma_start(oh[:], oh_in[:, uoff:uoff + nu, :])
                    for i, (ci, tt) in enumerate(sched_["units"][w]):
                        t = g * sched_["group_sz"] + tt
                        if t not in acc_tiles:
                            acc_tiles[t] = ap.tile(
                                [P, H], dt.float32,
                                tag=f"a{t % sched_['group_sz']}",
                                name=f"{prefix}acc_t{t}_w{w}")
                        st, sp = flags[(w, i)]
                        nc.tensor.matmul(
                            acc_tiles[t][:],
                            lhsT=oh[:, i, :],
                            rhs=msg[:, ci, par * H:(par + 1) * H],
                            start=st, stop=sp,
                        )

                def consume_edge(t, a):
                    # e = leaky(acc*dinv + G2); all-DVE to avoid cross-engine
                    # sem round-trips inside the serialized vector FIFO
                    s1 = postp.tile([P, H], dt.float32, tag="e1", name=f"e1_{t}")
                    nc.vector.tensor_scalar(
                        s1[:], a[:], dinv[:, t:t + 1], None, mybir.AluOpType.mult)
                    nc.vector.tensor_tensor(
                        s1[:], s1[:], g2_sb[:, t, :], mybir.AluOpType.add)
                    m = postp.tile([P, H], dt.float32, tag="m", name=f"m_{t}")
                    nc.vector.tensor_scalar(
                        m[:], s1[:], NEG, None, mybir.AluOpType.mult)
                    nc.vector.tensor_tensor(
                        e_sb[:, t, :], s1[:], m[:], mybir.AluOpType.max)

                with tc.tile_pool(name="eacc", bufs=1, space="PSUM") as accp:
                    epools = (msgp, ohp, accp)
                    ppools = (pmsgp, pohp, accp)

                    def emit_pair_group(pg):
                        acc_tiles = {}
                        for b in prot(pg):
                            emit_window(psched, pidx_sb, poh_in, eA_pairs,
                                        pflags, ppools, acc_tiles,
                                        pg * PNBUCKET + b, "p")
                        for t in range(pg * PGROUP, (pg + 1) * PGROUP):
                            nc.vector.tensor_copy(
                                xs_sb[:, t, :], acc_tiles.pop(t)[:])

                    for g in range(ngroups_e):
                        acc_tiles = {}
                        for b in erot(g):
                            emit_window(esched, eidx_sb, eoh_in, gA_pairs,
                                        eflags, epools, acc_tiles,
                                        g * NBUCKET + b, "e")
                        t0 = g * GROUP
                        t1 = min(t0 + GROUP, TILES)
                        for t in range(t0, t1):
                            consume_edge(t, acc_tiles.pop(t))
                        # stream this group's e rows out right away
                        nc.sync.dma_start(
                            e_shard[t0 * P:t1 * P, :].rearrange(
                                "(t p) f -> p t f", p=P),
                            e_sb[:, t0:t1, :],
                        )

                    nc.gpsimd.collective_compute(
                        "AllGather", mybir.AluOpType.bypass,
                        replica_groups=[list(range(NC))],
                        ins=[e_shard[:, :].opt()],
                        outs=[e_full[:, :].opt()],
                    )
                    for pg in range(ngroups_p):
                        emit_pair_group(pg)

                # ---------------- phase D: pair MLP head ----------------
                with (
                    tc.tile_pool(name="ptps", bufs=2, space="PSUM") as ptps,
                    tc.tile_pool(name="pzps", bufs=1, space="PSUM") as pzps,
                    tc.tile_pool(name="pops", bufs=1, space="PSUM") as pops,
                    tc.tile_pool(name="psb", bufs=2) as psbp,
                ):
                    for k in range(PCH):
                        xt_ps = ptps.tile([P, P], dt.float16)
                        nc.tensor.transpose(xt_ps[0:H, :], xs_sb[:, k, :], ident[:])
                        nc.tensor.transpose(xt_ps[H:P, :], xs_sb[:, PCH + k, :],
                                            ident[:])
                        xijt = psbp.tile([P, P], dt.float16, tag="xijt")
                        nc.vector.tensor_copy(xijt[:], xt_ps[:])
                        z_ps = pzps.tile([16, P], dt.float32)
                        nc.tensor.matmul(z_ps[:], lhsT=w1_sb[:], rhs=xijt[:],
                                         start=True, stop=True)
                        zb = psbp.tile([16, P], dt.float32, tag="zb")
                        nc.vector.tensor_scalar(
                            zb[:], z_ps[:], b1_sb[:, 0:1], None, mybir.AluOpType.add)
                        m2 = psbp.tile([16, P], dt.float32, tag="m2")
                        nc.vector.tensor_scalar(
                            m2[:], zb[:], NEG, None, mybir.AluOpType.mult)
                        z2 = psbp.tile([16, P], dt.float32, tag="z2")
                        nc.vector.tensor_tensor(z2[:], zb[:], m2[:],
                                                mybir.AluOpType.max)
                        o_ps = pops.tile([1, P], dt.float32)
                        nc.tensor.matmul(o_ps[:], lhsT=w2_sb[:], rhs=z2[:],
                                         start=True, stop=True)
                        osb = psbp.tile([1, P], dt.float32, tag="osb")
                        nc.scalar.activation(
                            osb[:], o_ps[:], mybir.ActivationFunctionType.Sigmoid,
                            bias=b2_sb[:, 0:1], scale=1.0)
                        nc.sync.dma_start(
                            outp[k * P:(k + 1) * P, :].rearrange("r one -> one r"),
                            osb[0:1, :])

    # align each gather's SWDGE queue with its Tile-assigned DMA lane so
    # semaphore<->queue locking stays consistent (4-way parallel desc gen)
    for blk in nc.m.functions[0].blocks:
        for inst in blk.instructions:
            if isinstance(inst, mybir.InstDMAGatherAnt):
                si = inst.sync_info
                for u in (si.on_update if si else []):
                    mm = re.match(r"DMASW(\d+)_", u.ant_name or "")
                    if mm:
                        inst.queue_num = int(mm.group(1)) % 4
                        break

    nc.compile()
    return nc


def kernel(**inputs) -> np.ndarray:
    in_maps, sched = _prep(inputs)
    nc = _build(sched)
    res = run_bass_kernel_spmd(nc, in_maps, list(range(NC)))
    out = np.concatenate([res.results[c]["out"] for c in range(NC)], axis=0)
    return out.astype(np.float32)
